# revision 26
# baseline (speedup 1.0000x reference)
"""Trainium2 Bass kernel for a dense transformer block.

Reference computation (per batch element):
    y  = Attention(LN1(x)) ; x = x + y
    x  = x + MLP(LN2(x))
with B=8, N=1024, C=768, H=12 heads, head_dim=64, HIDDEN=3072, fp32 I/O.

Sharding: data-parallel over B across the 8 NeuronCores — each core runs the
full block on one (1024, 768) batch element with replicated weights. No
collectives.

Per-core design notes (v4, query-split):
  * Attention is ACT-bound (softmax exp streams ~1 el/cycle/lane on the
    Scalar engine).  To keep the PE busy underneath it, the kernel splits
    attention by QUERY half: pass A covers queries 0-511 for all 6 head
    pairs (with QKV matmuls as PE filler), pass B covers queries 512-1023
    while the PE simultaneously runs the MLP of the first query half
    (proj, LN2, fc1) interleaved at key-tile granularity via a worklist.
  * LN rstd = exp(-0.5*ln(var+eps)) on the Scalar engine: Exp and Ln share
    one ACT table set ("natural_log_exp_and_others"), so LayerNorm can
    interleave with softmax exp without ~2.7us table reloads.  Gelu (its
    own table set) is deferred: fc1 in pass B evicts raw bf16, and the
    gelus run batched in pass C after the last exp.
  * Scores for a head pair are row-packed (even head rows 0-63, odd head
    rows 64-127) so the two K=64 matmuls run concurrently; exp is issued
    at free-dim 1024 (a 2-bank PSUM tile holding two key-tiles) writing
    fp8e4m3.
  * QKV / AV / proj / fc1 are fp8 DoubleRow matmuls (contraction 256 per
    instruction); fc2 stays bf16 (fp8 on both MLP matmuls would exceed the
    output tolerance).  fp8 weights are pre-scaled x8 on the host; the
    scales are folded into evictions, the AV denominator column (0.25 so
    attnT lands at x32), the proj residual, and the Gelu scale input.
  * Softmax reciprocal uses reciprocal_approx_fast; PSUM evictions run on
    the Vector engine.
"""

import collections

import numpy as np
import ml_dtypes

import concourse.bass as bass
import concourse.bacc as bacc
import concourse.mybir as mybir
import concourse.tile as tile
from concourse import bass_utils

# Model dims (hardcoded per the problem spec).
B = 8
N = 1024  # tokens
C = 768  # model dim
H = 12  # heads
HD = 64  # head dim
HID = 3072  # mlp hidden
EPS = 1e-5
P = 128  # SBUF partitions

NT = N // P  # 8 token tiles
KC = C // P  # 6 contraction tiles over C
KH = HID // P  # 24 contraction tiles over HIDDEN
NPAIR = H // 2  # 6 head pairs

F32 = mybir.dt.float32
BF16 = mybir.dt.bfloat16
FP8 = mybir.dt.float8e4
AF = mybir.ActivationFunctionType
ALU = mybir.AluOpType
DR = mybir.MatmulPerfMode.DoubleRow

# Feature switches (fallbacks for debugging).
ATT_FP8 = True  # fp8 DoubleRow qkv + attention@V + proj
FC1_FP8 = True  # fp8 DoubleRow fc1
FC2_FP8 = True  # fp8 DoubleRow fc2 (x8 weights; h stays true-scale)

WSCALE = 8.0  # host-side scale on fp8 weights

_cache = {}


def _build(flags):
    """Trace the per-core Bass program. `flags` gates optional bias/gain work."""
    (use_bqkv, use_g1, use_beta1, use_g2, use_beta2, use_bfc1, use_bproj,
     use_bfc2) = flags

    a_dt = FP8 if ATT_FP8 else BF16
    x2_dt = FP8 if FC1_FP8 else BF16
    w1_dt = FP8 if FC1_FP8 else BF16

    nc = bacc.Bacc("TRN2", target_bir_lowering=False, debug=False)

    x_d = nc.dram_tensor("x", [N, C], F32, kind="ExternalInput")
    wqkv_d = nc.dram_tensor("wqkv", [C, 3 * C], a_dt, kind="ExternalInput")
    wproj_d = nc.dram_tensor("wproj", [C, C], a_dt, kind="ExternalInput")
    wfc1_d = nc.dram_tensor("wfc1", [C, HID], w1_dt, kind="ExternalInput")
    w2_dt = FP8 if FC2_FP8 else BF16
    wfc2_d = nc.dram_tensor("wfc2", [HID, C], w2_dt, kind="ExternalInput")
    out_d = nc.dram_tensor("out", [N, C], F32, kind="ExternalOutput")

    opt_d = {}
    for name, use, shape in (
        ("bqkv", use_bqkv, [3 * C]),
        ("g1", use_g1, [C]),
        ("beta1", use_beta1, [C]),
        ("g2", use_g2, [C]),
        ("beta2", use_beta2, [C]),
        ("bfc1", use_bfc1, [HID]),
        ("bproj", use_bproj, [C]),
        ("bfc2", use_bfc2, [C]),
    ):
        if use:
            opt_d[name] = nc.dram_tensor(name, shape, F32, kind="ExternalInput")

    def bcast_from_dram(pool, ap_1d, n, name):
        """[n] DRAM vector -> [P, n] SBUF tile replicated on every partition."""
        t = pool.tile([P, n], F32, name=name)
        src = bass.AP(tensor=ap_1d.tensor, offset=ap_1d.offset,
                      ap=[[0, P]] + list(ap_1d.ap))
        nc.sync.dma_start(out=t, in_=src)
        return t

    with tile.TileContext(nc) as tc:
        persist = tc.alloc_tile_pool(name="persist", bufs=1, side="left")
        psum = tc.alloc_tile_pool(name="psum", bufs=1, space="PSUM")
        dram = tc.alloc_tile_pool(name="dram", bufs=2, space="DRAM")

        # Identity (bf16, embedded in the NEFF) for PE-based transposes and
        # ballast — loaded first so the HAM warmup burst can start early.
        ident_d = nc.inline_tensor(np.eye(P, dtype=ml_dtypes.bfloat16), "ident")
        ident = persist.tile([P, P], BF16)
        nc.sync.dma_start(out=ident, in_=ident_d.ap())

        # Residual stream, token-major; updated in place through the block.
        x_sb = persist.tile([P, NT, C], F32)
        for t in range(NT):
            nc.sync.dma_start(out=x_sb[:, t, :], in_=x_d.ap()[t * P:(t + 1) * P, :])
        eps_t = persist.tile([P, 1], F32)
        nc.vector.memset(eps_t, EPS)

        # fc1 weights + LN2 output live in persist so the wfc1 DMA can run
        # during attention instead of serializing after it (the dma_start is
        # emitted after wqkv/wproj so it doesn't delay them on the queue).
        wfc1_sb = persist.tile([P, KC, HID], w1_dt)
        x2lnT = persist.tile([P, KC, N], x2_dt)

        g_beta = {}
        for name in ("g1", "beta1", "g2", "beta2", "bproj", "bfc2"):
            if name in opt_d:
                g_beta[name] = bcast_from_dram(persist, opt_d[name].ap(), C,
                                               f"bc_{name}")
        bqkv_sb = None
        if "bqkv" in opt_d:
            bqkv_sb = persist.tile([P, 3 * C // P], F32)
            nc.sync.dma_start(out=bqkv_sb,
                              in_=opt_d["bqkv"].ap().rearrange("(m p) -> p m", p=P))
            g_beta["bqkv_v"] = bcast_from_dram(
                persist, opt_d["bqkv"].ap()[2 * C:3 * C], C, "bc_bqkv_v")
        bfc1_sb = None
        if "bfc1" in opt_d:
            bfc1_sb = persist.tile([P, KH], F32)
            nc.sync.dma_start(out=bfc1_sb,
                              in_=opt_d["bfc1"].ap().rearrange("(m p) -> p m", p=P))

        p1 = tc.alloc_tile_pool(name="p1", bufs=1, side="left")
        ln1 = tc.alloc_tile_pool(name="ln1", bufs=3, side="left")

        wqkv_sb = p1.tile([P, KC, 3 * C], a_dt)
        nc.sync.dma_start(out=wqkv_sb,
                          in_=wqkv_d.ap().rearrange("(k p) m -> p k m", p=P))

        xlnT = p1.tile([P, KC, N], a_dt)

        def ln_stats(pool, x_ap, name, bufs=3):
            """bn stats for one token tile -> [P, 2] (mean, var) tile."""
            stats = pool.tile([P, 3, 6], F32, tag=f"{name}_st", bufs=3)
            xr = x_ap.rearrange("p (s f) -> p s f", f=256)
            for s in range(3):
                nc.vector.bn_stats(out=stats[:, s, :], in_=xr[:, s, :])
            mv = pool.tile([P, 2], F32, tag=f"{name}_mv", bufs=bufs)
            nc.vector.bn_aggr(out=mv, in_=stats)
            return mv

        def ln_rstd(pool, mv, name, bufs=3):
            """[P,2] stats -> [P,1] rstd (ACT sqrt + DVE reciprocal)."""
            rstd = pool.tile([P, 1], F32, tag=f"{name}_rs", bufs=bufs)
            nc.scalar.activation(out=rstd, in_=mv[:, 1:2], func=AF.Sqrt,
                                 bias=eps_t, scale=1.0)
            nc.vector.reciprocal(out=rstd, in_=rstd)
            return rstd

        def ln_apply(pool, x_ap, mv, rstd, g_sb, beta_sb, name):
            xln = pool.tile([P, C], BF16, tag=f"{name}_xln", bufs=3)
            nc.vector.tensor_scalar(out=xln, in0=x_ap, scalar1=mv[:, 0:1],
                                    scalar2=rstd, op0=ALU.subtract, op1=ALU.mult)
            if g_sb is not None:
                nc.vector.tensor_mul(out=xln, in0=xln, in1=g_sb)
            if beta_sb is not None:
                nc.vector.tensor_add(out=xln, in0=xln, in1=beta_sb)
            return xln

        def layernorm_tile(pool, x_ap, g_sb, beta_sb, name):
            """x_ap: [P, C] fp32 token-major -> returns [P, C] bf16 tile."""
            mv = ln_stats(pool, x_ap, name)
            rstd = ln_rstd(pool, mv, name)
            return ln_apply(pool, x_ap, mv, rstd, g_sb, beta_sb, name)

        def transpose_to(xln, dstT, t, tag):
            """[P, C] token-major tile -> dstT[:, :, t*P:(t+1)*P] feature-major."""
            for c in range(KC):
                tps = psum.tile([P, P], BF16, tag=tag, bufs=2, name="tps")
                nc.tensor.transpose(tps, xln[:, c * P:(c + 1) * P], ident)
                nc.vector.tensor_copy(out=dstT[:, c, t * P:(t + 1) * P], in_=tps)

        p2 = tc.alloc_tile_pool(name="p2", bufs=1, side="right")
        att = tc.alloc_tile_pool(name="att", bufs=1, side="right")
        qkT = p2.tile([P, 2 * NPAIR, N], BF16)
        # V per head, token-tiles on dim2; slot HD is the denominator column
        # (0.25 with fp8 scaling so attnT comes out x32); slots HD+1.. pad
        # the kt stride to a multiple of 16 bytes (DoubleRow AP constraint).
        VW = 80 if ATT_FP8 else 72
        ONES = 0.25 if ATT_FP8 else 1.0
        v_aug = p2.tile([P, H, NT, VW], a_dt)
        nc.vector.memset(v_aug[:, :, :, HD:HD + 1], ONES)
        attnT = p2.tile([P, KC, N], a_dt)
        wproj_sb = p2.tile([P, KC, C], a_dt)
        nc.sync.dma_start(out=wproj_sb,
                          in_=wproj_d.ap().rearrange("(k p) m -> p k m", p=P))
        nc.sync.dma_start(out=wfc1_sb,
                          in_=wfc1_d.ap().rearrange("(k p) m -> p k m", p=P))

        es_tiles = {}
        fill_work = collections.deque()

        # Keep-warm ballast: the PE's HAM clock gate drops it to 1.2 GHz
        # after ~3.4us of low activity, which doubles every matmul in the
        # ACT-bound attention passes.  When there is no real PE work to
        # interleave, burn a couple of dummy matmuls to hold the 2.4 GHz
        # clock — they hide entirely under the Scalar engine's exp stream.
        warm_sb = persist.tile([P, 512], BF16)
        nc.vector.memset(warm_sb, 1.0)

        def ballast():
            bp = psum.tile([P, 512], F32, tag="mm", bufs=2, name="ballast")
            for r in range(2):
                nc.tensor.matmul(bp, ident, warm_sb,
                                 start=(r == 0), stop=(r == 1))

        def fill(k):
            for _ in range(k):
                if fill_work:
                    fill_work.popleft()()
                else:
                    ballast()

        def qkv_mms(ps, lhsT_of_ko, rhs_of_ko):
            """Contract over C with DoubleRow (fp8) or plain (bf16) matmuls."""
            if ATT_FP8:
                for kp in range(KC // 2):
                    nc.tensor.matmul(ps, lhsT_of_ko(2 * kp, 2),
                                     rhs_of_ko(2 * kp, 2),
                                     start=(kp == 0), stop=(kp == KC // 2 - 1),
                                     perf_mode=DR)
            else:
                for ko in range(KC):
                    nc.tensor.matmul(ps, lhsT_of_ko(ko, 1), rhs_of_ko(ko, 1),
                                     start=(ko == 0), stop=(ko == KC - 1))

        def emit_qk(p, hf):
            """q^T,k^T for pair p (bf16, pair-interleaved partitions)."""
            n0 = hf * 512
            for qk in range(2):
                m = qk * KC + p  # wqkv column block (q: 0-5, k: 6-11)
                ps = psum.tile([P, 512], F32, tag="mm", bufs=2, name="ps_qk")
                qkv_mms(ps,
                        lambda ko, kn: wqkv_sb[:, ko:ko + kn, m * P:(m + 1) * P],
                        lambda ko, kn: xlnT[:, ko:ko + kn, n0:n0 + 512])
                dst = qkT[:, qk * NPAIR + p, n0:n0 + 512]
                if bqkv_sb is not None:
                    nc.vector.tensor_scalar(
                        out=dst, in0=ps, scalar1=(1.0 / WSCALE) if ATT_FP8
                        else 1.0, scalar2=bqkv_sb[:, m:m + 1],
                        op0=ALU.mult, op1=ALU.add)
                elif ATT_FP8:
                    nc.vector.tensor_scalar_mul(dst, ps, 1.0 / WSCALE)
                else:
                    nc.vector.tensor_copy(out=dst, in_=ps)

        def emit_v(t):
            """V token-major for all heads, token tile t (x8 scale kept)."""
            for c0, nn, h0, nh in ((0, 512, 0, 8), (512, 256, 8, 4)):
                vps = psum.tile([P, 512], F32, tag="mm", bufs=2,
                                name="ps_v")[:, :nn]
                qkv_mms(vps,
                        lambda ko, kn: xlnT[:, ko:ko + kn, t * P:(t + 1) * P],
                        lambda ko, kn: wqkv_sb[:, ko:ko + kn,
                                              2 * C + c0:2 * C + c0 + nn])
                src = vps.rearrange("q (h d) -> q h d", d=HD)
                dst = v_aug[:, h0:h0 + nh, t, 0:HD]
                if bqkv_sb is not None:
                    bq = g_beta["bqkv_v"]
                    bsc = WSCALE if ATT_FP8 else 1.0
                    for hi in range(nh):
                        bs = bq[:, c0 + hi * HD:c0 + (hi + 1) * HD]
                        nc.vector.scalar_tensor_tensor(
                            out=dst[:, hi, :], in0=bs, scalar=bsc,
                            in1=src[:, hi, :], op0=ALU.mult, op1=ALU.add)
                else:
                    # Scalar engine: it is idle during the prologue while
                    # the Vector engine is saturated by LN1.
                    nc.scalar.copy(out=dst, in_=src)

        def emit_scores_kt(p, kt, j):
            """Row-packed score matmuls + exp for one key tile of pair p.

            One [P,512] PSUM tile per (head, key tile), 4 in rotation: the
            exp queue always has ~2 instructions of slack, so the Scalar
            engine streams without waiting on the scores lockstep.
            """
            scs = []
            for hh in range(2):
                scs.append(psum.tile([P, 512], F32, tag="sc", bufs=4,
                                     name=f"sc{hh}"))
            for hh in range(2):
                pb = hh * HD
                nc.tensor.matmul(
                    scs[hh],
                    qkT[pb:pb + HD, NPAIR + p, kt * P:(kt + 1) * P],
                    qkT[pb:pb + HD, p, j * 512:(j + 1) * 512],
                    start=True, stop=True)
            for hh in range(2):
                nc.scalar.activation(out=es_tiles[2 * p + hh][:, kt, :],
                                     in_=scs[hh], func=AF.Exp, scale=0.125)

        def emit_av(p, j):
            """AV (+denominator row) for both heads of pair p, then normalize."""
            for hh in range(2):
                h = 2 * p + hh
                es = es_tiles[h]
                av = psum.tile([HD + 1, 512], F32, tag="av", bufs=2,
                               name=f"av{hh}")
                if ATT_FP8:
                    for ktp in range(NT // 2):
                        nc.tensor.matmul(
                            av, v_aug[:, h, 2 * ktp:2 * ktp + 2, 0:HD + 1],
                            es[:, 2 * ktp:2 * ktp + 2, :],
                            start=(ktp == 0), stop=(ktp == NT // 2 - 1),
                            perf_mode=DR)
                else:
                    for kt in range(NT):
                        nc.tensor.matmul(av, v_aug[:, h, kt, 0:HD + 1],
                                         es[:, kt, :],
                                         start=(kt == 0), stop=(kt == NT - 1))
                av_sb = att.tile([HD + 1, 512], F32, tag="avsb", bufs=2,
                                 name=f"avsb{hh}")
                nc.vector.tensor_copy(out=av_sb, in_=av)
                # Softmax denominators: row HD holds ONES*sum_k exp(S).
                # Broadcast via a DRAM bounce, reciprocal, normalize.
                rdram = dram.tile([1, 512], F32, tag="rdram", bufs=2)
                nc.gpsimd.dma_start(out=rdram, in_=av_sb[HD:HD + 1, :])
                rbc = att.tile([HD, 512], F32, tag="rbc", bufs=2,
                               name=f"rbc{hh}")
                rd = rdram[0, :]
                rbc_src = bass.AP(tensor=rd.tensor, offset=rd.offset,
                                  ap=[[0, HD]] + list(rd.ap))
                nc.gpsimd.dma_start(out=rbc, in_=rbc_src)
                nc.vector.reciprocal_approx_fast(out=rbc, in_=rbc)
                js = slice(j * 512, (j + 1) * 512)
                if hh == 0:
                    nc.vector.tensor_mul(out=attnT[0:HD, p, js],
                                         in0=av_sb[0:HD, :], in1=rbc)
                else:
                    bounce = att.tile([HD, 512], a_dt, tag="bounce", bufs=2,
                                      name="bounce")
                    nc.vector.tensor_mul(out=bounce, in0=av_sb[0:HD, :],
                                         in1=rbc)
                    nc.gpsimd.dma_start(out=attnT[HD:P, p, js], in_=bounce)

        def alloc_es(p):
            for hh in range(2):
                es_tiles[2 * p + hh] = att.tile([P, NT, 512], FP8 if ATT_FP8
                                                else BF16, tag="es", bufs=4,
                                                name=f"es{2 * p + hh}")

        # ---------------------------------------------------------------
        # Prologue: LN1 + transposes, V for every token tile, QKV(0).
        # A ballast burst up front engages the HAM clock gate before the
        # real matmuls start (PE transposes don't count as PE activity).
        # ---------------------------------------------------------------
        for _ in range(10):
            ballast()
        for t in range(NT):
            xln = layernorm_tile(ln1, x_sb[:, t, :], g_beta.get("g1"),
                                 g_beta.get("beta1"), "ln1")
            transpose_to(xln, xlnT, t, "av")
            emit_v(t)
            ballast()
            ballast()
            if t == 3 or t == 7:
                if t == 3:
                    alloc_es(0)
                emit_qk(0, (t - 3) // 4)

        # ---------------------------------------------------------------
        # Pass A: attention for queries 0-511, QKV matmuls as PE filler.
        # ---------------------------------------------------------------
        for i in range(1, NPAIR + 2):
            if i >= 2:
                emit_av(i - 2, 0)
            if i < NPAIR:
                alloc_es(i)
                for hf in range(2):
                    fill_work.append(
                        lambda p=i, hf=hf: emit_qk(p, hf))
            if i <= NPAIR:
                for kt in range(NT):
                    emit_scores_kt(i - 1, kt, 0)
                    if kt % 2 == 1:
                        fill(1)
                    if kt % 4 == 2:
                        ballast()
        while fill_work:
            fill(1)

        ln1.release()
        p1.release()

        # ---------------------------------------------------------------
        # Pass B: attention for queries 512-1023; PE filler = proj + LN2 +
        # raw fc1 (gelu deferred) for the first query half.
        # ---------------------------------------------------------------
        ln2 = tc.alloc_tile_pool(name="ln2", bufs=3, side="left")
        p5 = tc.alloc_tile_pool(name="p5", bufs=1, side="right")
        hT = p5.tile([P, KH, N], FP8 if FC2_FP8 else BF16)
        # raw (pre-gelu) fc1 outputs for the first query half: gelu runs
        # later, batched, to keep its table set out of the exp stream.
        hT_raw = p5.tile([P, KH, 512], BF16)
        wfc2_sb = p5.tile([P, KH, C], w2_dt)
        nc.sync.dma_start(out=wfc2_sb,
                          in_=wfc2_d.ap().rearrange("(k p) m -> p k m", p=P))

        def emit_proj(t):
            for n0, nn in ((0, 512), (512, 256)):
                ps = psum.tile([P, 512], F32, tag="mm", bufs=2,
                               name="ps_pj")[:, :nn]
                qkv_mms(ps,
                        lambda ko, kn: attnT[:, ko:ko + kn, t * P:(t + 1) * P],
                        lambda ko, kn: wproj_sb[:, ko:ko + kn, n0:n0 + nn])
                xs = x_sb[:, t, n0:n0 + nn]
                if ATT_FP8:
                    nc.vector.scalar_tensor_tensor(
                        out=xs, in0=ps, scalar=1.0 / (32 * WSCALE), in1=xs,
                        op0=ALU.mult, op1=ALU.add)
                else:
                    nc.vector.tensor_add(out=xs, in0=xs, in1=ps)
                if "bproj" in g_beta:
                    nc.vector.tensor_add(out=xs, in0=xs,
                                         in1=g_beta["bproj"][:, n0:n0 + nn])

        # LN2 is split so the ACT sqrt ops can run in two batches (one table
        # switch each) instead of thrashing table sets against softmax exp.
        ln2_mv = {}
        ln2_rstd = {}

        def emit_ln2_stats(t):
            ln2_mv[t] = ln_stats(ln2, x_sb[:, t, :], "ln2", bufs=9)

        def emit_rstd_batch(ts):
            for t in ts:
                ln2_rstd[t] = ln_rstd(ln2, ln2_mv[t], "ln2", bufs=9)

        def emit_ln2_apply(t):
            xln = ln_apply(ln2, x_sb[:, t, :], ln2_mv[t], ln2_rstd[t],
                           g_beta.get("g2"), g_beta.get("beta2"), "ln2")
            transpose_to(xln, x2lnT, t, "mm")

        def emit_fc1_raw(m, n0):
            """fc1 matmul for block m, half n0; evict raw bf16 (no gelu)."""
            ps = psum.tile([P, 512], F32, tag="mm", bufs=2, name="ps_f1")
            if FC1_FP8:
                for kp in range(KC // 2):
                    nc.tensor.matmul(
                        ps, wfc1_sb[:, 2 * kp:2 * kp + 2, m * P:(m + 1) * P],
                        x2lnT[:, 2 * kp:2 * kp + 2, n0:n0 + 512],
                        start=(kp == 0), stop=(kp == KC // 2 - 1),
                        perf_mode=DR)
            else:
                for ko in range(KC):
                    nc.tensor.matmul(ps, wfc1_sb[:, ko, m * P:(m + 1) * P],
                                     x2lnT[:, ko, n0:n0 + 512],
                                     start=(ko == 0), stop=(ko == KC - 1))
            nc.vector.tensor_copy(out=hT_raw[:, m, :], in_=ps)

        gelu_scale = (1.0 / WSCALE) if FC1_FP8 else 1.0

        def emit_gelu_inplace(m, n0):
            bias = bfc1_sb[:, m:m + 1] if bfc1_sb is not None else 0.0
            nc.scalar.activation(out=hT[:, m, n0:n0 + 512],
                                 in_=hT_raw[:, m, :], func=AF.Gelu,
                                 bias=bias, scale=gelu_scale)

        def emit_fc1_gelu(m, n0):
            """fc1 matmul + direct gelu (pass C: no exp stream to fight)."""
            ps = psum.tile([P, 512], F32, tag="mm", bufs=2, name="ps_f1")
            if FC1_FP8:
                for kp in range(KC // 2):
                    nc.tensor.matmul(
                        ps, wfc1_sb[:, 2 * kp:2 * kp + 2, m * P:(m + 1) * P],
                        x2lnT[:, 2 * kp:2 * kp + 2, n0:n0 + 512],
                        start=(kp == 0), stop=(kp == KC // 2 - 1),
                        perf_mode=DR)
            else:
                for ko in range(KC):
                    nc.tensor.matmul(ps, wfc1_sb[:, ko, m * P:(m + 1) * P],
                                     x2lnT[:, ko, n0:n0 + 512],
                                     start=(ko == 0), stop=(ko == KC - 1))
            bias = bfc1_sb[:, m:m + 1] if bfc1_sb is not None else 0.0
            nc.scalar.activation(out=hT[:, m, n0:n0 + 512], in_=ps,
                                 func=AF.Gelu, bias=bias, scale=gelu_scale)

        def emit_fc2(t):
            for n0, nn in ((0, 512), (512, 256)):
                ps = psum.tile([P, 512], F32, tag="mm", bufs=2,
                               name="ps_f2")[:, :nn]
                if FC2_FP8:
                    for kp in range(KH // 2):
                        nc.tensor.matmul(
                            ps, hT[:, 2 * kp:2 * kp + 2, t * P:(t + 1) * P],
                            wfc2_sb[:, 2 * kp:2 * kp + 2, n0:n0 + nn],
                            start=(kp == 0), stop=(kp == KH // 2 - 1),
                            perf_mode=DR)
                else:
                    for ko in range(KH):
                        nc.tensor.matmul(ps, hT[:, ko, t * P:(t + 1) * P],
                                         wfc2_sb[:, ko, n0:n0 + nn],
                                         start=(ko == 0), stop=(ko == KH - 1))
                xs = x_sb[:, t, n0:n0 + nn]
                if FC2_FP8:
                    nc.vector.scalar_tensor_tensor(
                        out=xs, in0=ps, scalar=1.0 / WSCALE, in1=xs,
                        op0=ALU.mult, op1=ALU.add)
                else:
                    nc.vector.tensor_add(out=xs, in0=xs, in1=ps)
                if "bfc2" in g_beta:
                    nc.vector.tensor_add(out=xs, in0=xs,
                                         in1=g_beta["bfc2"][:, n0:n0 + nn])
            nc.sync.dma_start(out=out_d.ap()[t * P:(t + 1) * P, :],
                              in_=x_sb[:, t, :])

        # Worklist for pass B (dependency-ordered).  Leading ballast: the
        # first proj depends on the last pass-A normalize chain (~8us of
        # DMA latency); dummy matmuls keep the PE stream from blocking.
        for _ in range(4):
            fill_work.append(ballast)
        fill_work.append(lambda: emit_proj(0))
        fill_work.append(lambda: emit_proj(1))
        fill_work.append(lambda: emit_ln2_stats(0))
        fill_work.append(lambda: emit_proj(2))
        fill_work.append(lambda: emit_ln2_stats(1))
        fill_work.append(lambda: emit_proj(3))
        fill_work.append(lambda: emit_ln2_stats(2))
        fill_work.append(lambda: emit_ln2_stats(3))
        fill_work.append(lambda: emit_rstd_batch(range(4)))
        for t in range(4):
            fill_work.append(lambda t=t: emit_ln2_apply(t))
        for m in range(KH):
            fill_work.append(lambda m=m: emit_fc1_raw(m, 0))

        for p in range(NPAIR):
            if p >= 1:
                emit_av(p - 1, 1)
            alloc_es(p)
            for kt in range(NT):
                emit_scores_kt(p, kt, 1)
                if kt % 2 == 1:
                    fill(1)
        emit_av(NPAIR - 1, 1)
        # Drain the remaining fc1 closures interleaved with their gelus:
        # all exps are done, so the gelu table set loads once and the
        # Scalar engine rolls straight on instead of idling behind the
        # drain.  fill_work is ordered, so when gelu m is emitted its
        # fc1_raw m writer has already been emitted.
        for m in range(KH):
            fill(1)
            emit_gelu_inplace(m, 0)
        while fill_work:
            fill(1)
        # The first pass-C matmuls wait on the last normalize chain; keep
        # the PE (and its clock) busy across that ~10us dependency gap.
        for _ in range(12):
            ballast()

        # ---------------------------------------------------------------
        # Pass C: second-half MLP + gelus + fc2 + output.  The LN2 sqrt
        # batch goes on the ACT queue before the 48 gelus so the fc1-n1
        # matmuls aren't stuck behind them.
        # ---------------------------------------------------------------
        for t in range(4, NT):
            emit_proj(t)
            emit_ln2_stats(t)
        emit_rstd_batch(range(4, NT))
        for t in range(4, NT):
            emit_ln2_apply(t)
        for m in range(KH):
            emit_fc1_gelu(m, 512)
        for t in range(NT):
            emit_fc2(t)

        ln2.release()
        p5.release()
        att.release()
        p2.release()
        persist.release()
        dram.release()
        psum.release()

    nc.compile()
    return nc


def _prep(inputs):
    """Host-side prep: shard x over B, cast weights, compute flag gates."""
    f = {k: np.asarray(v) for k, v in inputs.items()}
    bf = ml_dtypes.bfloat16
    f8 = getattr(ml_dtypes, "float8_e4m3fn", None) or ml_dtypes.float8_e4m3

    flags = (
        bool(np.any(f["b_qkv"])),
        not np.all(f["g1"] == 1.0),
        bool(np.any(f["beta1"])),
        not np.all(f["g2"] == 1.0),
        bool(np.any(f["beta2"])),
        bool(np.any(f["b_fc1"])),
        bool(np.any(f["b_proj"])),
        bool(np.any(f["b_fc2"])),
    )
    (use_bqkv, use_g1, use_beta1, use_g2, use_beta2, use_bfc1, use_bproj,
     use_bfc2) = flags

    def wcast(w, fp8_on):
        if fp8_on:
            return np.ascontiguousarray((w * WSCALE).astype(f8))
        return np.ascontiguousarray(w.astype(bf))

    common = {
        "wqkv": wcast(f["w_qkv"], ATT_FP8),
        "wproj": wcast(f["w_proj"], ATT_FP8),
        "wfc1": wcast(f["w_fc1"], FC1_FP8),
        "wfc2": wcast(f["w_fc2"], FC2_FP8),
    }
    for name, key, use in (
        ("bqkv", "b_qkv", use_bqkv), ("g1", "g1", use_g1),
        ("beta1", "beta1", use_beta1), ("g2", "g2", use_g2),
        ("beta2", "beta2", use_beta2), ("bfc1", "b_fc1", use_bfc1),
        ("bproj", "b_proj", use_bproj), ("bfc2", "b_fc2", use_bfc2),
    ):
        if use:
            common[name] = np.ascontiguousarray(f[key].astype(np.float32))

    x = f["x"].astype(np.float32)
    in_maps = [dict(common, x=np.ascontiguousarray(x[i])) for i in range(B)]
    return flags, in_maps


LAST_RESULT = None


def kernel(**inputs):
    global LAST_RESULT
    flags, in_maps = _prep(inputs)
    if flags not in _cache:
        _cache[flags] = _build(flags)
    nc = _cache[flags]
    res = bass_utils.run_bass_kernel_spmd(nc, in_maps, core_ids=list(range(B)))
    LAST_RESULT = res
    out = np.stack([r["out"] for r in res.results], axis=0)
    return out.astype(np.float32)


# revision 27
# speedup vs baseline: 1.0193x; 1.0193x over previous
"""Trainium2 Bass kernel for a dense transformer block.

Reference computation (per batch element):
    y  = Attention(LN1(x)) ; x = x + y
    x  = x + MLP(LN2(x))
with B=8, N=1024, C=768, H=12 heads, head_dim=64, HIDDEN=3072, fp32 I/O.

Sharding: data-parallel over B across the 8 NeuronCores — each core runs the
full block on one (1024, 768) batch element with replicated weights. No
collectives.

Per-core design notes (v4, query-split):
  * Attention is ACT-bound (softmax exp streams ~1 el/cycle/lane on the
    Scalar engine).  To keep the PE busy underneath it, the kernel splits
    attention by QUERY half: pass A covers queries 0-511 for all 6 head
    pairs (with QKV matmuls as PE filler), pass B covers queries 512-1023
    while the PE simultaneously runs the MLP of the first query half
    (proj, LN2, fc1) interleaved at key-tile granularity via a worklist.
  * LN rstd = exp(-0.5*ln(var+eps)) on the Scalar engine: Exp and Ln share
    one ACT table set ("natural_log_exp_and_others"), so LayerNorm can
    interleave with softmax exp without ~2.7us table reloads.  Gelu (its
    own table set) is deferred: fc1 in pass B evicts raw bf16, and the
    gelus run batched in pass C after the last exp.
  * Scores for a head pair are row-packed (even head rows 0-63, odd head
    rows 64-127) so the two K=64 matmuls run concurrently; exp is issued
    at free-dim 1024 (a 2-bank PSUM tile holding two key-tiles) writing
    fp8e4m3.
  * QKV / AV / proj / fc1 are fp8 DoubleRow matmuls (contraction 256 per
    instruction); fc2 stays bf16 (fp8 on both MLP matmuls would exceed the
    output tolerance).  fp8 weights are pre-scaled x8 on the host; the
    scales are folded into evictions, the AV denominator column (0.25 so
    attnT lands at x32), the proj residual, and the Gelu scale input.
  * Softmax reciprocal uses reciprocal_approx_fast; PSUM evictions run on
    the Vector engine.
"""

import collections

import numpy as np
import ml_dtypes

import concourse.bass as bass
import concourse.bacc as bacc
import concourse.mybir as mybir
import concourse.tile as tile
from concourse import bass_utils

# Model dims (hardcoded per the problem spec).
B = 8
N = 1024  # tokens
C = 768  # model dim
H = 12  # heads
HD = 64  # head dim
HID = 3072  # mlp hidden
EPS = 1e-5
P = 128  # SBUF partitions

NT = N // P  # 8 token tiles
KC = C // P  # 6 contraction tiles over C
KH = HID // P  # 24 contraction tiles over HIDDEN
NPAIR = H // 2  # 6 head pairs

F32 = mybir.dt.float32
BF16 = mybir.dt.bfloat16
FP8 = mybir.dt.float8e4
AF = mybir.ActivationFunctionType
ALU = mybir.AluOpType
DR = mybir.MatmulPerfMode.DoubleRow

# Feature switches (fallbacks for debugging).
ATT_FP8 = True  # fp8 DoubleRow qkv + attention@V + proj
FC1_FP8 = True  # fp8 DoubleRow fc1
FC2_FP8 = True  # fp8 DoubleRow fc2 (x8 weights; h stays true-scale)

WSCALE = 8.0  # host-side scale on fp8 weights

_cache = {}


def _build(flags):
    """Trace the per-core Bass program. `flags` gates optional bias/gain work."""
    (use_bqkv, use_g1, use_beta1, use_g2, use_beta2, use_bfc1, use_bproj,
     use_bfc2) = flags

    a_dt = FP8 if ATT_FP8 else BF16
    x2_dt = FP8 if FC1_FP8 else BF16
    w1_dt = FP8 if FC1_FP8 else BF16

    nc = bacc.Bacc("TRN2", target_bir_lowering=False, debug=False)

    x_d = nc.dram_tensor("x", [N, C], F32, kind="ExternalInput")
    wqkv_d = nc.dram_tensor("wqkv", [C, 3 * C], a_dt, kind="ExternalInput")
    wproj_d = nc.dram_tensor("wproj", [C, C], a_dt, kind="ExternalInput")
    wfc1_d = nc.dram_tensor("wfc1", [C, HID], w1_dt, kind="ExternalInput")
    w2_dt = FP8 if FC2_FP8 else BF16
    wfc2_d = nc.dram_tensor("wfc2", [HID, C], w2_dt, kind="ExternalInput")
    out_d = nc.dram_tensor("out", [N, C], F32, kind="ExternalOutput")

    opt_d = {}
    for name, use, shape in (
        ("bqkv", use_bqkv, [3 * C]),
        ("g1", use_g1, [C]),
        ("beta1", use_beta1, [C]),
        ("g2", use_g2, [C]),
        ("beta2", use_beta2, [C]),
        ("bfc1", use_bfc1, [HID]),
        ("bproj", use_bproj, [C]),
        ("bfc2", use_bfc2, [C]),
    ):
        if use:
            opt_d[name] = nc.dram_tensor(name, shape, F32, kind="ExternalInput")

    def bcast_from_dram(pool, ap_1d, n, name):
        """[n] DRAM vector -> [P, n] SBUF tile replicated on every partition."""
        t = pool.tile([P, n], F32, name=name)
        src = bass.AP(tensor=ap_1d.tensor, offset=ap_1d.offset,
                      ap=[[0, P]] + list(ap_1d.ap))
        nc.sync.dma_start(out=t, in_=src)
        return t

    with tile.TileContext(nc) as tc:
        persist = tc.alloc_tile_pool(name="persist", bufs=1, side="left")
        psum = tc.alloc_tile_pool(name="psum", bufs=1, space="PSUM")
        dram = tc.alloc_tile_pool(name="dram", bufs=2, space="DRAM")

        # Identity (bf16, embedded in the NEFF) for PE-based transposes and
        # ballast — loaded first so the HAM warmup burst can start early.
        ident_d = nc.inline_tensor(np.eye(P, dtype=ml_dtypes.bfloat16), "ident")
        ident = persist.tile([P, P], BF16)
        nc.sync.dma_start(out=ident, in_=ident_d.ap())

        # Residual stream, token-major; updated in place through the block.
        x_sb = persist.tile([P, NT, C], F32)
        for t in range(NT):
            nc.sync.dma_start(out=x_sb[:, t, :], in_=x_d.ap()[t * P:(t + 1) * P, :])
        eps_t = persist.tile([P, 1], F32)
        nc.vector.memset(eps_t, EPS)

        # fc1 weights + LN2 output live in persist so the wfc1 DMA can run
        # during attention instead of serializing after it (the dma_start is
        # emitted after wqkv/wproj so it doesn't delay them on the queue).
        wfc1_sb = persist.tile([P, KC, HID], w1_dt)
        x2lnT = persist.tile([P, KC, N], x2_dt)

        g_beta = {}
        for name in ("g1", "beta1", "g2", "beta2", "bproj", "bfc2"):
            if name in opt_d:
                g_beta[name] = bcast_from_dram(persist, opt_d[name].ap(), C,
                                               f"bc_{name}")
        bqkv_sb = None
        if "bqkv" in opt_d:
            bqkv_sb = persist.tile([P, 3 * C // P], F32)
            nc.sync.dma_start(out=bqkv_sb,
                              in_=opt_d["bqkv"].ap().rearrange("(m p) -> p m", p=P))
            g_beta["bqkv_v"] = bcast_from_dram(
                persist, opt_d["bqkv"].ap()[2 * C:3 * C], C, "bc_bqkv_v")
        bfc1_sb = None
        if "bfc1" in opt_d:
            bfc1_sb = persist.tile([P, KH], F32)
            nc.sync.dma_start(out=bfc1_sb,
                              in_=opt_d["bfc1"].ap().rearrange("(m p) -> p m", p=P))

        p1 = tc.alloc_tile_pool(name="p1", bufs=1, side="left")
        ln1 = tc.alloc_tile_pool(name="ln1", bufs=3, side="left")

        wqkv_sb = p1.tile([P, KC, 3 * C], a_dt)
        nc.sync.dma_start(out=wqkv_sb,
                          in_=wqkv_d.ap().rearrange("(k p) m -> p k m", p=P))

        xlnT = p1.tile([P, KC, N], a_dt)

        def ln_stats(pool, x_ap, name, bufs=3):
            """bn stats for one token tile -> [P, 2] (mean, var) tile."""
            stats = pool.tile([P, 3, 6], F32, tag=f"{name}_st", bufs=3)
            xr = x_ap.rearrange("p (s f) -> p s f", f=256)
            for s in range(3):
                nc.vector.bn_stats(out=stats[:, s, :], in_=xr[:, s, :])
            mv = pool.tile([P, 2], F32, tag=f"{name}_mv", bufs=bufs)
            nc.vector.bn_aggr(out=mv, in_=stats)
            return mv

        def ln_rstd(pool, mv, name, bufs=3):
            """[P,2] stats -> [P,1] rstd (ACT sqrt + DVE reciprocal)."""
            rstd = pool.tile([P, 1], F32, tag=f"{name}_rs", bufs=bufs)
            nc.scalar.activation(out=rstd, in_=mv[:, 1:2], func=AF.Sqrt,
                                 bias=eps_t, scale=1.0)
            nc.vector.reciprocal(out=rstd, in_=rstd)
            return rstd

        def ln_apply(pool, x_ap, mv, rstd, g_sb, beta_sb, name):
            xln = pool.tile([P, C], BF16, tag=f"{name}_xln", bufs=3)
            nc.vector.tensor_scalar(out=xln, in0=x_ap, scalar1=mv[:, 0:1],
                                    scalar2=rstd, op0=ALU.subtract, op1=ALU.mult)
            if g_sb is not None:
                nc.vector.tensor_mul(out=xln, in0=xln, in1=g_sb)
            if beta_sb is not None:
                nc.vector.tensor_add(out=xln, in0=xln, in1=beta_sb)
            return xln

        def layernorm_tile(pool, x_ap, g_sb, beta_sb, name):
            """x_ap: [P, C] fp32 token-major -> returns [P, C] bf16 tile."""
            mv = ln_stats(pool, x_ap, name)
            rstd = ln_rstd(pool, mv, name)
            return ln_apply(pool, x_ap, mv, rstd, g_sb, beta_sb, name)

        def transpose_to(xln, dstT, t, tag):
            """[P, C] token-major tile -> dstT[:, :, t*P:(t+1)*P] feature-major."""
            for c in range(KC):
                tps = psum.tile([P, P], BF16, tag=tag, bufs=2, name="tps")
                nc.tensor.transpose(tps, xln[:, c * P:(c + 1) * P], ident)
                nc.vector.tensor_copy(out=dstT[:, c, t * P:(t + 1) * P], in_=tps)

        p2 = tc.alloc_tile_pool(name="p2", bufs=1, side="right")
        att = tc.alloc_tile_pool(name="att", bufs=1, side="right")
        qkT = p2.tile([P, 2 * NPAIR, N], BF16)
        # V per head, token-tiles on dim2; slot HD is the denominator column
        # (0.25 with fp8 scaling so attnT comes out x32); slots HD+1.. pad
        # the kt stride to a multiple of 16 bytes (DoubleRow AP constraint).
        VW = 80 if ATT_FP8 else 72
        ONES = 0.25 if ATT_FP8 else 1.0
        v_aug = p2.tile([P, H, NT, VW], a_dt)
        nc.vector.memset(v_aug[:, :, :, HD:HD + 1], ONES)
        attnT = p2.tile([P, KC, N], a_dt)
        wproj_sb = p2.tile([P, KC, C], a_dt)
        nc.sync.dma_start(out=wproj_sb,
                          in_=wproj_d.ap().rearrange("(k p) m -> p k m", p=P))
        nc.sync.dma_start(out=wfc1_sb,
                          in_=wfc1_d.ap().rearrange("(k p) m -> p k m", p=P))

        es_tiles = {}
        fill_work = collections.deque()

        # Keep-warm ballast: the PE's HAM clock gate drops it to 1.2 GHz
        # after ~3.4us of low activity, which doubles every matmul in the
        # ACT-bound attention passes.  When there is no real PE work to
        # interleave, burn a couple of dummy matmuls to hold the 2.4 GHz
        # clock — they hide entirely under the Scalar engine's exp stream.
        warm_sb = persist.tile([P, 512], BF16)
        nc.vector.memset(warm_sb, 1.0)

        def ballast():
            bp = psum.tile([P, 512], F32, tag="mm", bufs=2, name="ballast")
            for r in range(2):
                nc.tensor.matmul(bp, ident, warm_sb,
                                 start=(r == 0), stop=(r == 1))

        def fill(k):
            for _ in range(k):
                if fill_work:
                    fill_work.popleft()()
                else:
                    ballast()

        def qkv_mms(ps, lhsT_of_ko, rhs_of_ko):
            """Contract over C with DoubleRow (fp8) or plain (bf16) matmuls."""
            if ATT_FP8:
                for kp in range(KC // 2):
                    nc.tensor.matmul(ps, lhsT_of_ko(2 * kp, 2),
                                     rhs_of_ko(2 * kp, 2),
                                     start=(kp == 0), stop=(kp == KC // 2 - 1),
                                     perf_mode=DR)
            else:
                for ko in range(KC):
                    nc.tensor.matmul(ps, lhsT_of_ko(ko, 1), rhs_of_ko(ko, 1),
                                     start=(ko == 0), stop=(ko == KC - 1))

        def emit_qk(p, hf):
            """q^T,k^T for pair p (bf16, pair-interleaved partitions)."""
            n0 = hf * 512
            for qk in range(2):
                m = qk * KC + p  # wqkv column block (q: 0-5, k: 6-11)
                ps = psum.tile([P, 512], F32, tag="mm", bufs=2, name="ps_qk")
                qkv_mms(ps,
                        lambda ko, kn: wqkv_sb[:, ko:ko + kn, m * P:(m + 1) * P],
                        lambda ko, kn: xlnT[:, ko:ko + kn, n0:n0 + 512])
                dst = qkT[:, qk * NPAIR + p, n0:n0 + 512]
                if bqkv_sb is not None:
                    nc.vector.tensor_scalar(
                        out=dst, in0=ps, scalar1=(1.0 / WSCALE) if ATT_FP8
                        else 1.0, scalar2=bqkv_sb[:, m:m + 1],
                        op0=ALU.mult, op1=ALU.add)
                elif ATT_FP8:
                    nc.vector.tensor_scalar_mul(dst, ps, 1.0 / WSCALE)
                else:
                    nc.vector.tensor_copy(out=dst, in_=ps)

        def emit_v(t):
            """V token-major for all heads, token tile t (x8 scale kept)."""
            for c0, nn, h0, nh in ((0, 512, 0, 8), (512, 256, 8, 4)):
                vps = psum.tile([P, 512], F32, tag="mm", bufs=2,
                                name="ps_v")[:, :nn]
                qkv_mms(vps,
                        lambda ko, kn: xlnT[:, ko:ko + kn, t * P:(t + 1) * P],
                        lambda ko, kn: wqkv_sb[:, ko:ko + kn,
                                              2 * C + c0:2 * C + c0 + nn])
                src = vps.rearrange("q (h d) -> q h d", d=HD)
                dst = v_aug[:, h0:h0 + nh, t, 0:HD]
                if bqkv_sb is not None:
                    bq = g_beta["bqkv_v"]
                    bsc = WSCALE if ATT_FP8 else 1.0
                    for hi in range(nh):
                        bs = bq[:, c0 + hi * HD:c0 + (hi + 1) * HD]
                        nc.vector.scalar_tensor_tensor(
                            out=dst[:, hi, :], in0=bs, scalar=bsc,
                            in1=src[:, hi, :], op0=ALU.mult, op1=ALU.add)
                else:
                    # Scalar engine: it is idle during the prologue while
                    # the Vector engine is saturated by LN1.
                    nc.scalar.copy(out=dst, in_=src)

        def emit_scores_kt(p, kt, j):
            """Row-packed score matmuls + exp for one key tile of pair p.

            One [P,512] PSUM tile per (head, key tile), 4 in rotation: the
            exp queue always has ~2 instructions of slack, so the Scalar
            engine streams without waiting on the scores lockstep.
            """
            scs = []
            for hh in range(2):
                scs.append(psum.tile([P, 512], F32, tag="sc", bufs=4,
                                     name=f"sc{hh}"))
            for hh in range(2):
                pb = hh * HD
                nc.tensor.matmul(
                    scs[hh],
                    qkT[pb:pb + HD, NPAIR + p, kt * P:(kt + 1) * P],
                    qkT[pb:pb + HD, p, j * 512:(j + 1) * 512],
                    start=True, stop=True)
            for hh in range(2):
                nc.scalar.activation(out=es_tiles[2 * p + hh][:, kt, :],
                                     in_=scs[hh], func=AF.Exp, scale=0.125)

        def emit_av(p, j):
            """AV (+denominator row) for both heads of pair p, then normalize."""
            for hh in range(2):
                h = 2 * p + hh
                es = es_tiles[h]
                av = psum.tile([HD + 1, 512], F32, tag="av", bufs=2,
                               name=f"av{hh}")
                if ATT_FP8:
                    for ktp in range(NT // 2):
                        nc.tensor.matmul(
                            av, v_aug[:, h, 2 * ktp:2 * ktp + 2, 0:HD + 1],
                            es[:, 2 * ktp:2 * ktp + 2, :],
                            start=(ktp == 0), stop=(ktp == NT // 2 - 1),
                            perf_mode=DR)
                else:
                    for kt in range(NT):
                        nc.tensor.matmul(av, v_aug[:, h, kt, 0:HD + 1],
                                         es[:, kt, :],
                                         start=(kt == 0), stop=(kt == NT - 1))
                av_sb = att.tile([HD + 1, 512], F32, tag="avsb", bufs=2,
                                 name=f"avsb{hh}")
                nc.vector.tensor_copy(out=av_sb, in_=av)
                # Softmax denominators: row HD holds ONES*sum_k exp(S).
                # Broadcast via a DRAM bounce, reciprocal, normalize.
                rdram = dram.tile([1, 512], F32, tag="rdram", bufs=2)
                nc.gpsimd.dma_start(out=rdram, in_=av_sb[HD:HD + 1, :])
                rbc = att.tile([HD, 512], F32, tag="rbc", bufs=2,
                               name=f"rbc{hh}")
                rd = rdram[0, :]
                rbc_src = bass.AP(tensor=rd.tensor, offset=rd.offset,
                                  ap=[[0, HD]] + list(rd.ap))
                nc.gpsimd.dma_start(out=rbc, in_=rbc_src)
                nc.vector.reciprocal_approx_fast(out=rbc, in_=rbc)
                js = slice(j * 512, (j + 1) * 512)
                if hh == 0:
                    nc.vector.tensor_mul(out=attnT[0:HD, p, js],
                                         in0=av_sb[0:HD, :], in1=rbc)
                else:
                    bounce = att.tile([HD, 512], a_dt, tag="bounce", bufs=2,
                                      name="bounce")
                    nc.vector.tensor_mul(out=bounce, in0=av_sb[0:HD, :],
                                         in1=rbc)
                    nc.gpsimd.dma_start(out=attnT[HD:P, p, js], in_=bounce)

        def alloc_es(p):
            for hh in range(2):
                es_tiles[2 * p + hh] = att.tile([P, NT, 512], FP8 if ATT_FP8
                                                else BF16, tag="es", bufs=4,
                                                name=f"es{2 * p + hh}")

        # ---------------------------------------------------------------
        # Prologue: LN1 + transposes, V for every token tile, QKV(0).
        # A ballast burst up front engages the HAM clock gate before the
        # real matmuls start (PE transposes don't count as PE activity).
        # ---------------------------------------------------------------
        for _ in range(10):
            ballast()
        for t in range(NT):
            xln = layernorm_tile(ln1, x_sb[:, t, :], g_beta.get("g1"),
                                 g_beta.get("beta1"), "ln1")
            transpose_to(xln, xlnT, t, "av")
            emit_v(t)
            ballast()
            ballast()
            if t == 3 or t == 7:
                if t == 3:
                    alloc_es(0)
                emit_qk(0, (t - 3) // 4)

        # ---------------------------------------------------------------
        # Pass A: attention for queries 0-511, QKV matmuls as PE filler.
        # ---------------------------------------------------------------
        for i in range(1, NPAIR + 2):
            if i >= 2:
                emit_av(i - 2, 0)
            if i < NPAIR:
                alloc_es(i)
                for hf in range(2):
                    fill_work.append(
                        lambda p=i, hf=hf: emit_qk(p, hf))
            if i <= NPAIR:
                for kt in range(NT):
                    emit_scores_kt(i - 1, kt, 0)
                    if kt % 2 == 1:
                        fill(1)
                    if kt % 4 == 2:
                        ballast()
        while fill_work:
            fill(1)

        ln1.release()
        p1.release()

        # ---------------------------------------------------------------
        # Pass B: attention for queries 512-1023; PE filler = proj + LN2 +
        # raw fc1 (gelu deferred) for the first query half.
        # ---------------------------------------------------------------
        ln2 = tc.alloc_tile_pool(name="ln2", bufs=3, side="left")
        p5 = tc.alloc_tile_pool(name="p5", bufs=1, side="right")
        hT = p5.tile([P, KH, N], FP8 if FC2_FP8 else BF16)
        # raw (pre-gelu) fc1 outputs for the first query half: gelu runs
        # later, batched, to keep its table set out of the exp stream.
        hT_raw = p5.tile([P, KH, 512], BF16)
        wfc2_sb = p5.tile([P, KH, C], w2_dt)
        nc.sync.dma_start(out=wfc2_sb,
                          in_=wfc2_d.ap().rearrange("(k p) m -> p k m", p=P))

        def emit_proj(t):
            for n0, nn in ((0, 512), (512, 256)):
                ps = psum.tile([P, 512], F32, tag="mm", bufs=2,
                               name="ps_pj")[:, :nn]
                qkv_mms(ps,
                        lambda ko, kn: attnT[:, ko:ko + kn, t * P:(t + 1) * P],
                        lambda ko, kn: wproj_sb[:, ko:ko + kn, n0:n0 + nn])
                xs = x_sb[:, t, n0:n0 + nn]
                if ATT_FP8:
                    nc.vector.scalar_tensor_tensor(
                        out=xs, in0=ps, scalar=1.0 / (32 * WSCALE), in1=xs,
                        op0=ALU.mult, op1=ALU.add)
                else:
                    nc.vector.tensor_add(out=xs, in0=xs, in1=ps)
                if "bproj" in g_beta:
                    nc.vector.tensor_add(out=xs, in0=xs,
                                         in1=g_beta["bproj"][:, n0:n0 + nn])

        # LN2 is split so the ACT sqrt ops can run in two batches (one table
        # switch each) instead of thrashing table sets against softmax exp.
        ln2_mv = {}
        ln2_rstd = {}

        def emit_ln2_stats(t):
            ln2_mv[t] = ln_stats(ln2, x_sb[:, t, :], "ln2", bufs=9)

        def emit_rstd_batch(ts):
            for t in ts:
                ln2_rstd[t] = ln_rstd(ln2, ln2_mv[t], "ln2", bufs=9)

        def emit_ln2_apply(t):
            xln = ln_apply(ln2, x_sb[:, t, :], ln2_mv[t], ln2_rstd[t],
                           g_beta.get("g2"), g_beta.get("beta2"), "ln2")
            transpose_to(xln, x2lnT, t, "mm")

        def emit_fc1_raw(m, n0):
            """fc1 matmul for block m, half n0; evict raw bf16 (no gelu)."""
            ps = psum.tile([P, 512], F32, tag="mm", bufs=2, name="ps_f1")
            if FC1_FP8:
                for kp in range(KC // 2):
                    nc.tensor.matmul(
                        ps, wfc1_sb[:, 2 * kp:2 * kp + 2, m * P:(m + 1) * P],
                        x2lnT[:, 2 * kp:2 * kp + 2, n0:n0 + 512],
                        start=(kp == 0), stop=(kp == KC // 2 - 1),
                        perf_mode=DR)
            else:
                for ko in range(KC):
                    nc.tensor.matmul(ps, wfc1_sb[:, ko, m * P:(m + 1) * P],
                                     x2lnT[:, ko, n0:n0 + 512],
                                     start=(ko == 0), stop=(ko == KC - 1))
            nc.vector.tensor_copy(out=hT_raw[:, m, :], in_=ps)

        gelu_scale = (1.0 / WSCALE) if FC1_FP8 else 1.0

        def emit_gelu_inplace(m, n0):
            bias = bfc1_sb[:, m:m + 1] if bfc1_sb is not None else 0.0
            nc.scalar.activation(out=hT[:, m, n0:n0 + 512],
                                 in_=hT_raw[:, m, :], func=AF.Gelu,
                                 bias=bias, scale=gelu_scale)

        def emit_fc1_gelu(m, n0):
            """fc1 matmul + direct gelu (pass C: no exp stream to fight)."""
            ps = psum.tile([P, 512], F32, tag="mm", bufs=2, name="ps_f1")
            if FC1_FP8:
                for kp in range(KC // 2):
                    nc.tensor.matmul(
                        ps, wfc1_sb[:, 2 * kp:2 * kp + 2, m * P:(m + 1) * P],
                        x2lnT[:, 2 * kp:2 * kp + 2, n0:n0 + 512],
                        start=(kp == 0), stop=(kp == KC // 2 - 1),
                        perf_mode=DR)
            else:
                for ko in range(KC):
                    nc.tensor.matmul(ps, wfc1_sb[:, ko, m * P:(m + 1) * P],
                                     x2lnT[:, ko, n0:n0 + 512],
                                     start=(ko == 0), stop=(ko == KC - 1))
            bias = bfc1_sb[:, m:m + 1] if bfc1_sb is not None else 0.0
            nc.scalar.activation(out=hT[:, m, n0:n0 + 512], in_=ps,
                                 func=AF.Gelu, bias=bias, scale=gelu_scale)

        def emit_fc2(t):
            for n0, nn in ((0, 512), (512, 256)):
                ps = psum.tile([P, 512], F32, tag="mm", bufs=2,
                               name="ps_f2")[:, :nn]
                if FC2_FP8:
                    for kp in range(KH // 2):
                        nc.tensor.matmul(
                            ps, hT[:, 2 * kp:2 * kp + 2, t * P:(t + 1) * P],
                            wfc2_sb[:, 2 * kp:2 * kp + 2, n0:n0 + nn],
                            start=(kp == 0), stop=(kp == KH // 2 - 1),
                            perf_mode=DR)
                else:
                    for ko in range(KH):
                        nc.tensor.matmul(ps, hT[:, ko, t * P:(t + 1) * P],
                                         wfc2_sb[:, ko, n0:n0 + nn],
                                         start=(ko == 0), stop=(ko == KH - 1))
                xs = x_sb[:, t, n0:n0 + nn]
                if FC2_FP8:
                    nc.vector.scalar_tensor_tensor(
                        out=xs, in0=ps, scalar=1.0 / WSCALE, in1=xs,
                        op0=ALU.mult, op1=ALU.add)
                else:
                    nc.vector.tensor_add(out=xs, in0=xs, in1=ps)
                if "bfc2" in g_beta:
                    nc.vector.tensor_add(out=xs, in0=xs,
                                         in1=g_beta["bfc2"][:, n0:n0 + nn])
            nc.sync.dma_start(out=out_d.ap()[t * P:(t + 1) * P, :],
                              in_=x_sb[:, t, :])

        # Worklist for pass B (dependency-ordered).  Leading ballast: the
        # first proj depends on the last pass-A normalize chain (~8us of
        # DMA latency); dummy matmuls keep the PE stream from blocking.
        for _ in range(4):
            fill_work.append(ballast)
        fill_work.append(lambda: emit_proj(0))
        fill_work.append(lambda: emit_proj(1))
        fill_work.append(lambda: emit_ln2_stats(0))
        fill_work.append(lambda: emit_proj(2))
        fill_work.append(lambda: emit_ln2_stats(1))
        fill_work.append(lambda: emit_proj(3))
        fill_work.append(lambda: emit_ln2_stats(2))
        fill_work.append(lambda: emit_ln2_stats(3))
        fill_work.append(lambda: emit_rstd_batch(range(4)))
        for t in range(4):
            fill_work.append(lambda t=t: emit_ln2_apply(t))
        for m in range(KH):
            fill_work.append(lambda m=m: emit_fc1_raw(m, 0))

        for p in range(NPAIR):
            if p >= 1:
                emit_av(p - 1, 1)
            alloc_es(p)
            for kt in range(NT):
                emit_scores_kt(p, kt, 1)
                if kt % 2 == 1:
                    fill(1)
        emit_av(NPAIR - 1, 1)
        while fill_work:
            fill(1)
        # The first pass-C matmuls wait on the last normalize chain; keep
        # the PE (and its clock) busy across that ~10us dependency gap.
        for _ in range(12):
            ballast()

        # ---------------------------------------------------------------
        # Pass C: second-half MLP + gelus + fc2 + output.  The LN2 sqrt
        # batch goes on the ACT queue before the 48 gelus so the fc1-n1
        # matmuls aren't stuck behind them.
        # ---------------------------------------------------------------
        for t in range(4, NT):
            emit_proj(t)
            emit_ln2_stats(t)
        emit_rstd_batch(range(4, NT))
        for t in range(4, NT):
            emit_ln2_apply(t)
        for m in range(KH):
            emit_gelu_inplace(m, 0)
        for m in range(KH):
            emit_fc1_gelu(m, 512)
        for t in range(NT):
            emit_fc2(t)

        ln2.release()
        p5.release()
        att.release()
        p2.release()
        persist.release()
        dram.release()
        psum.release()

    nc.compile()
    return nc


def _prep(inputs):
    """Host-side prep: shard x over B, cast weights, compute flag gates."""
    f = {k: np.asarray(v) for k, v in inputs.items()}
    bf = ml_dtypes.bfloat16
    f8 = getattr(ml_dtypes, "float8_e4m3fn", None) or ml_dtypes.float8_e4m3

    flags = (
        bool(np.any(f["b_qkv"])),
        not np.all(f["g1"] == 1.0),
        bool(np.any(f["beta1"])),
        not np.all(f["g2"] == 1.0),
        bool(np.any(f["beta2"])),
        bool(np.any(f["b_fc1"])),
        bool(np.any(f["b_proj"])),
        bool(np.any(f["b_fc2"])),
    )
    (use_bqkv, use_g1, use_beta1, use_g2, use_beta2, use_bfc1, use_bproj,
     use_bfc2) = flags

    def wcast(w, fp8_on):
        if fp8_on:
            return np.ascontiguousarray((w * WSCALE).astype(f8))
        return np.ascontiguousarray(w.astype(bf))

    common = {
        "wqkv": wcast(f["w_qkv"], ATT_FP8),
        "wproj": wcast(f["w_proj"], ATT_FP8),
        "wfc1": wcast(f["w_fc1"], FC1_FP8),
        "wfc2": wcast(f["w_fc2"], FC2_FP8),
    }
    for name, key, use in (
        ("bqkv", "b_qkv", use_bqkv), ("g1", "g1", use_g1),
        ("beta1", "beta1", use_beta1), ("g2", "g2", use_g2),
        ("beta2", "beta2", use_beta2), ("bfc1", "b_fc1", use_bfc1),
        ("bproj", "b_proj", use_bproj), ("bfc2", "b_fc2", use_bfc2),
    ):
        if use:
            common[name] = np.ascontiguousarray(f[key].astype(np.float32))

    x = f["x"].astype(np.float32)
    in_maps = [dict(common, x=np.ascontiguousarray(x[i])) for i in range(B)]
    return flags, in_maps


LAST_RESULT = None


def kernel(**inputs):
    global LAST_RESULT
    flags, in_maps = _prep(inputs)
    if flags not in _cache:
        _cache[flags] = _build(flags)
    nc = _cache[flags]
    res = bass_utils.run_bass_kernel_spmd(nc, in_maps, core_ids=list(range(B)))
    LAST_RESULT = res
    out = np.stack([r["out"] for r in res.results], axis=0)
    return out.astype(np.float32)
